# revision 29
# baseline (speedup 1.0000x reference)
"""Trainium2 Bass kernel for nn_MoELayer_90984587198472 (moe_routing).

The reference computes a dense all-expert MoE [E,B,S,O] einsum, but its
(faithfully reproduced) gather bug indexes the feature dim with the top-k
slot index j in {0,1}:  sel[b,s,j] = all_out[top_idx[b,s,j], b, s, j].
So only output columns 0 and 1 of each expert's projection matter, and the
whole computation collapses to a 24-column matmul per token:

  cols 0..7  : gate logits  x @ gate_W.T          (+ gate_b + expert_biases)
  cols 8..15 : expert col 0 x @ W_experts[:,0,:].T (+ b_experts[:,0])
  cols 16..23: expert col 1 x @ W_experts[:,1,:].T (+ b_experts[:,1])

then: p = sigmoid(gate), (m1,i1),(m2,i2) = top2(p),
      combined = (m1*v0[i1] + m2*v1[i2]) / (m1+m2),
      final[b,s,:] = combined (broadcast over O), top_idx = (i1,i2).

Sharding: data-parallel over tokens. B*S = 8192 tokens split as 1024 per
core across 8 cores; the 24x1024 fused weight slice is replicated. The
x shard is uploaded in [D, tok] (transposed, tile-major) layout so the
TensorEngine can contract over D directly (PE requires the contraction
dim on partitions); all arithmetic stays on device.

Device dataflow per token block (progressive sizes 128/256/.../128):
  DMA in  xb [128, GT, 8, 128]  (contiguous per partition)
  PE      y.T [24, GT*128] += wct_c.T @ xb_c  over 8 chunks
  DVE     copy y.T PSUM -> SBUF
  PE      back-transpose [24,128] -> [128,24] per 128-token tile
  DVE     + bias; sigmoid (ACT); hw top-8 max/max_index; masks; gather;
          normalize -> combined
  ACT/DVE broadcast combined across O=1024 (alternating tiles)
  DMA out final block + idx int32 at the end
"""

import numpy as np
from contextlib import ExitStack

import concourse.bass as bass
import concourse.bacc as bacc
import concourse.tile as tile
from concourse import mybir
from concourse.bass_utils import run_bass_kernel_spmd
from concourse.masks import make_identity

F32 = mybir.dt.float32
I32 = mybir.dt.int32
AX = mybir.AxisListType
OP = mybir.AluOpType
ACT = mybir.ActivationFunctionType

N_CORES = 8
B, S, D, O, E = 4, 2048, 1024, 1024, 8
TOK = B * S // N_CORES      # tokens per core = 1024
P = 128                     # partitions
NT = TOK // P               # 128-token tiles per core = 8
KC = D // P                 # contraction chunks = 8
M = 3 * E                   # fused matmul output columns = 24
BLOCKS = [1, 1, 2, 2, 1, 1]  # tiles per block: small first block (PE ramps
NB = len(BLOCKS)             # early) and small last blocks (short tail)


def build_bass():
    nc = bacc.Bacc()
    xt = nc.declare_dram_parameter("xt", [P, NT, KC, P], F32, isOutput=False)
    wct = nc.declare_dram_parameter("wct", [P, KC, M], F32, isOutput=False)
    biast = nc.declare_dram_parameter("biast", [P, M], F32, isOutput=False)
    final = nc.declare_dram_parameter("final", [TOK, O], F32, isOutput=True)
    idx = nc.declare_dram_parameter("idx", [TOK, 2], I32, isOutput=True)

    with tile.TileContext(nc) as tc, ExitStack() as ctx:
        consts = ctx.enter_context(tc.tile_pool(name="consts", bufs=1))
        xin = ctx.enter_context(tc.tile_pool(name="xin", bufs=6))
        psy = ctx.enter_context(tc.tile_pool(name="psy", bufs=3, space="PSUM"))
        pst = ctx.enter_context(tc.tile_pool(name="pst", bufs=3, space="PSUM"))
        small = ctx.enter_context(tc.tile_pool(name="small", bufs=3))
        outs = ctx.enter_context(tc.tile_pool(name="outs", bufs=3))

        # --- constants ---
        ident = consts.tile([P, P], F32)
        make_identity(nc, ident)
        ones = consts.tile([P, O], F32)
        nc.vector.memset(ones, 1.0)
        iota_i = consts.tile([P, 1, 2, E], I32)
        nc.gpsimd.iota(iota_i[:, 0, :, :], pattern=[[0, 2], [1, E]], base=0,
                       channel_multiplier=0)
        iota_f = consts.tile([P, 1, 2, E], F32)
        nc.vector.tensor_copy(iota_f, iota_i)
        wct_sb = consts.tile([P, KC, M], F32)
        nc.gpsimd.dma_start(out=wct_sb, in_=wct[:])
        bias_sb = consts.tile([P, M], F32)
        nc.gpsimd.dma_start(out=bias_sb, in_=biast[:])
        idx_all = consts.tile([P, NT, 2], I32)

        # Warm the PE instruction stream during setup: the PE otherwise sits
        # idle through the preamble and pays a ~4us IRAM fetch stall on its
        # first real matmul. These transposes depend only on the identity.
        warm = ctx.enter_context(tc.tile_pool(name="warm", bufs=1,
                                              space="PSUM"))
        wpt = warm.tile([P, P], F32)
        for _ in range(4):
            nc.tensor.transpose(wpt, ident, ident)

        xt_view = xt[:]
        ft_view = final[:].rearrange("(t p) d -> p t d", p=P)

        ts = 0
        for b, GT in enumerate(BLOCKS):
            TB = GT * P
            xb = xin.tile([P, GT, KC, P], F32, tag="xb")
            nc.sync.dma_start(out=xb, in_=xt_view[:, ts:ts + GT])

            # y.T [24, TB] accumulated over KC chunks; weights stationary
            yt = psy.tile([M, TB], F32, tag="yt")
            for c in range(KC):
                nc.tensor.matmul(yt, lhsT=wct_sb[:, c, :], rhs=xb[:, :, c, :],
                                 start=(c == 0), stop=(c == KC - 1))
            ysb = small.tile([M, TB], F32, tag="ysb")
            nc.scalar.copy(ysb, yt)

            # back-transpose each 128-token slice to [128, 24] and add bias
            yg = small.tile([P, GT, M], F32, tag="yg")
            for t in range(GT):
                ptt = pst.tile([P, M], F32)
                nc.tensor.transpose(ptt, ysb[:, t * P:(t + 1) * P],
                                    ident[0:M, 0:M])
                nc.vector.tensor_add(yg[:, t, :], ptt, bias_sb)

            # --- per-token routing math ---
            probs = small.tile([P, GT, E], F32, tag="probs")
            nc.scalar.activation(probs, yg[:, :, 0:E], ACT.Sigmoid)
            # HW top-8: values (descending) + first-occurrence indices
            vals = small.tile([P, GT, E], F32, tag="vals")
            midx = small.tile([P, GT, E], mybir.dt.uint32, tag="midx")
            for t in range(GT):
                nc.vector.max(out=vals[:, t, :], in_=probs[:, t, :])
                nc.vector.max_index(out=midx[:, t, :], in_max=vals[:, t, :],
                                    in_values=probs[:, t, :])
            # fused 16-wide gather: wv[k,e] = (e==idx_k) * m_k * v_k[e]
            sh4 = [P, GT, 2, E]
            idxf = small.tile([P, GT, 2, 1], F32, tag="idxf")
            nc.vector.tensor_copy(idxf[:, :, :, 0], midx[:, :, 0:2])
            nc.vector.tensor_copy(idx_all[:, ts:ts + GT, :],
                                  midx[:, :, 0:2])
            mask12 = small.tile(sh4, F32, tag="mask12")
            nc.vector.tensor_tensor(mask12, iota_f.to_broadcast(sh4),
                                    idxf.to_broadcast(sh4), OP.is_equal)
            mval = small.tile(sh4, F32, tag="mval")
            nc.vector.tensor_tensor(
                mval, mask12,
                vals[:, :, 0:2].to_broadcast(sh4),
                OP.mult)
            wv = small.tile(sh4, F32, tag="wv")
            nc.vector.tensor_tensor(
                wv, mval,
                yg[:, :, E:3 * E].rearrange("p g (k e) -> p g k e", e=E),
                OP.mult)
            num = small.tile([P, GT, 1], F32, tag="num")
            nc.vector.tensor_reduce(num, wv, AX.XY, OP.add)
            # combined = num / (m1+m2)
            den = small.tile([P, GT, 1], F32, tag="den")
            nc.vector.tensor_tensor(den, vals[:, :, 0:1], vals[:, :, 1:2],
                                    OP.add)
            rec = small.tile([P, GT, 1], F32, tag="rec")
            nc.vector.reciprocal(rec, den)
            comb = small.tile([P, GT, 1], F32, tag="comb")
            nc.vector.tensor_tensor(comb, num, rec, OP.mult)

            # broadcast combined across O and store per tile: each tile's
            # 0.5 MB ships as soon as its broadcast finishes instead of
            # waiting for the whole block
            out_g = outs.tile([P, GT, O], F32)
            for t in range(GT):
                gt_i = ts + t
                if gt_i % 2 == 0:
                    nc.scalar.mul(out_g[:, t, :], ones, comb[:, t, :])
                else:
                    nc.vector.tensor_scalar_mul(out_g[:, t, :], ones,
                                                comb[:, t, :])
                o_eng = nc.scalar if gt_i % 2 == 0 else nc.sync
                o_eng.dma_start(out=ft_view[:, gt_i], in_=out_g[:, t, :])
            ts += GT

        nc.gpsimd.dma_start(
            out=idx[:].rearrange("(t p) k -> p t k", p=P), in_=idx_all)
    nc.finalize()
    return nc


def _prep_host_inputs(x, W_experts, b_experts, gate_W, gate_b, expert_biases):
    xf = np.asarray(x, dtype=np.float32).reshape(-1, D)
    Wc = np.concatenate([gate_W, W_experts[:, 0, :], W_experts[:, 1, :]],
                        axis=0).astype(np.float32)         # [24, D]
    # wct[p, c, m] = Wc[m, c*128 + p]
    wct = np.ascontiguousarray(
        Wc.T.reshape(KC, P, M).transpose(1, 0, 2)).astype(np.float32)
    bias24 = np.concatenate(
        [gate_b + expert_biases, b_experts[:, 0], b_experts[:, 1]]
    ).astype(np.float32)
    biast = np.ascontiguousarray(np.tile(bias24[None, :], (P, 1)))
    return xf, wct, biast


def make_in_maps(x, W_experts, b_experts, gate_W, gate_b, expert_biases):
    xf, wct, biast = _prep_host_inputs(
        x, W_experts, b_experts, gate_W, gate_b, expert_biases)
    maps = []
    for c in range(N_CORES):
        xT = xf[c * TOK:(c + 1) * TOK].T              # [D, TOK]
        x5 = xT.reshape(KC, P, NT, P).transpose(1, 2, 0, 3)
        maps.append({"xt": np.ascontiguousarray(x5),
                     "wct": wct, "biast": biast})
    return maps


def kernel(x, W_experts, b_experts, gate_W, gate_b, expert_biases):
    x = np.asarray(x, dtype=np.float32)
    W_experts = np.asarray(W_experts, dtype=np.float32)
    b_experts = np.asarray(b_experts, dtype=np.float32)
    gate_W = np.asarray(gate_W, dtype=np.float32)
    gate_b = np.asarray(gate_b, dtype=np.float32)
    expert_biases = np.asarray(expert_biases, dtype=np.float32)

    in_maps = make_in_maps(
        x, W_experts, b_experts, gate_W, gate_b, expert_biases)
    nc = build_bass()
    res = run_bass_kernel_spmd(nc, in_maps, list(range(N_CORES))).results
    final = np.concatenate([res[c]["final"] for c in range(N_CORES)], axis=0)
    idx = np.concatenate([res[c]["idx"] for c in range(N_CORES)], axis=0)
    return (final.reshape(B, S, O),
            idx.reshape(B, S, 2).astype(np.int32))


# revision 30
# speedup vs baseline: 1.0875x; 1.0875x over previous
"""Trainium2 Bass kernel for nn_MoELayer_90984587198472 (moe_routing).

The reference computes a dense all-expert MoE [E,B,S,O] einsum, but its
(faithfully reproduced) gather bug indexes the feature dim with the top-k
slot index j in {0,1}:  sel[b,s,j] = all_out[top_idx[b,s,j], b, s, j].
So only output columns 0 and 1 of each expert's projection matter, and the
whole computation collapses to a 24-column matmul per token:

  cols 0..7  : gate logits  x @ gate_W.T          (+ gate_b + expert_biases)
  cols 8..15 : expert col 0 x @ W_experts[:,0,:].T (+ b_experts[:,0])
  cols 16..23: expert col 1 x @ W_experts[:,1,:].T (+ b_experts[:,1])

then: p = sigmoid(gate), (m1,i1),(m2,i2) = top2(p),
      combined = (m1*v0[i1] + m2*v1[i2]) / (m1+m2),
      final[b,s,:] = combined (broadcast over O), top_idx = (i1,i2).

Sharding: data-parallel over tokens. B*S = 8192 tokens split as 1024 per
core across 8 cores; the 24x1024 fused weight slice is replicated. The
x shard is uploaded in [D, tok] (transposed, tile-major) layout so the
TensorEngine can contract over D directly (PE requires the contraction
dim on partitions); all arithmetic stays on device.

Device dataflow per token block (progressive sizes 128/256/.../128):
  DMA in  xb [128, GT, 8, 128]  (contiguous per partition)
  PE      y.T [24, GT*128] += wct_c.T @ xb_c  over 8 chunks
  DVE     copy y.T PSUM -> SBUF
  PE      back-transpose [24,128] -> [128,24] per 128-token tile
  DVE     + bias; sigmoid (ACT); hw top-8 max/max_index; masks; gather;
          normalize -> combined
  ACT/DVE broadcast combined across O=1024 (alternating tiles)
  DMA out final block + idx int32 at the end
"""

import numpy as np
from contextlib import ExitStack

import concourse.bass as bass
import concourse.bacc as bacc
import concourse.tile as tile
from concourse import mybir
from concourse.bass_utils import run_bass_kernel_spmd
from concourse.masks import make_identity

F32 = mybir.dt.float32
I32 = mybir.dt.int32
AX = mybir.AxisListType
OP = mybir.AluOpType
ACT = mybir.ActivationFunctionType

N_CORES = 8
B, S, D, O, E = 4, 2048, 1024, 1024, 8
TOK = B * S // N_CORES      # tokens per core = 1024
P = 128                     # partitions
NT = TOK // P               # 128-token tiles per core = 8
KC = D // P                 # contraction chunks = 8
M = 3 * E                   # fused matmul output columns = 24
BLOCKS = [1, 2, 2, 1, 1, 1]  # tiles per block: small first block (PE ramps
NB = len(BLOCKS)             # early) and small last blocks (short tail)


def build_bass():
    nc = bacc.Bacc()
    xt = nc.declare_dram_parameter("xt", [P, NT, KC, P], F32, isOutput=False)
    wct = nc.declare_dram_parameter("wct", [P, KC, M], F32, isOutput=False)
    biast = nc.declare_dram_parameter("biast", [P, M], F32, isOutput=False)
    final = nc.declare_dram_parameter("final", [TOK, O], F32, isOutput=True)
    idx = nc.declare_dram_parameter("idx", [TOK, 2], I32, isOutput=True)

    with tile.TileContext(nc) as tc, ExitStack() as ctx:
        consts = ctx.enter_context(tc.tile_pool(name="consts", bufs=1))
        xin = ctx.enter_context(tc.tile_pool(name="xin", bufs=6))
        psy = ctx.enter_context(tc.tile_pool(name="psy", bufs=3, space="PSUM"))
        pst = ctx.enter_context(tc.tile_pool(name="pst", bufs=3, space="PSUM"))
        small = ctx.enter_context(tc.tile_pool(name="small", bufs=3))
        outs = ctx.enter_context(tc.tile_pool(name="outs", bufs=3))

        # --- constants ---
        ident = consts.tile([P, P], F32)
        make_identity(nc, ident)
        ones = consts.tile([P, O], F32)
        nc.vector.memset(ones, 1.0)
        iota_i = consts.tile([P, 1, 2, E], I32)
        nc.gpsimd.iota(iota_i[:, 0, :, :], pattern=[[0, 2], [1, E]], base=0,
                       channel_multiplier=0)
        iota_f = consts.tile([P, 1, 2, E], F32)
        nc.vector.tensor_copy(iota_f, iota_i)
        wct_sb = consts.tile([P, KC, M], F32)
        nc.gpsimd.dma_start(out=wct_sb, in_=wct[:])
        bias_sb = consts.tile([P, M], F32)
        nc.gpsimd.dma_start(out=bias_sb, in_=biast[:])
        idx_all = consts.tile([P, NT, 2], I32)

        # Warm the PE instruction stream during setup: the PE otherwise sits
        # idle through the preamble and pays a ~4us IRAM fetch stall on its
        # first real matmul. These transposes depend only on the identity.
        warm = ctx.enter_context(tc.tile_pool(name="warm", bufs=1,
                                              space="PSUM"))
        wpt = warm.tile([P, P], F32)
        for _ in range(4):
            nc.tensor.transpose(wpt, ident, ident)

        xt_view = xt[:]
        ft_view = final[:].rearrange("(t p) d -> p t d", p=P)

        ts = 0
        for b, GT in enumerate(BLOCKS):
            TB = GT * P
            xb = xin.tile([P, GT, KC, P], F32, tag="xb")
            nc.sync.dma_start(out=xb, in_=xt_view[:, ts:ts + GT])

            # y.T [24, TB] accumulated over KC chunks; weights stationary
            yt = psy.tile([M, TB], F32, tag="yt")
            for c in range(KC):
                nc.tensor.matmul(yt, lhsT=wct_sb[:, c, :], rhs=xb[:, :, c, :],
                                 start=(c == 0), stop=(c == KC - 1))
            ysb = small.tile([M, TB], F32, tag="ysb")
            nc.scalar.copy(ysb, yt)

            # back-transpose each 128-token slice to [128, 24] and add bias
            yg = small.tile([P, GT, M], F32, tag="yg")
            for t in range(GT):
                ptt = pst.tile([P, M], F32)
                nc.tensor.transpose(ptt, ysb[:, t * P:(t + 1) * P],
                                    ident[0:M, 0:M])
                nc.vector.tensor_add(yg[:, t, :], ptt, bias_sb)

            # --- per-token routing math ---
            probs = small.tile([P, GT, E], F32, tag="probs")
            nc.scalar.activation(probs, yg[:, :, 0:E], ACT.Sigmoid)
            # HW top-8: values (descending) + first-occurrence indices
            vals = small.tile([P, GT, E], F32, tag="vals")
            midx = small.tile([P, GT, E], mybir.dt.uint32, tag="midx")
            for t in range(GT):
                nc.vector.max(out=vals[:, t, :], in_=probs[:, t, :])
                nc.vector.max_index(out=midx[:, t, :], in_max=vals[:, t, :],
                                    in_values=probs[:, t, :])
            # fused 16-wide gather: wv[k,e] = (e==idx_k) * m_k * v_k[e]
            sh4 = [P, GT, 2, E]
            idxf = small.tile([P, GT, 2, 1], F32, tag="idxf")
            nc.vector.tensor_copy(idxf[:, :, :, 0], midx[:, :, 0:2])
            nc.vector.tensor_copy(idx_all[:, ts:ts + GT, :],
                                  midx[:, :, 0:2])
            mask12 = small.tile(sh4, F32, tag="mask12")
            nc.vector.tensor_tensor(mask12, iota_f.to_broadcast(sh4),
                                    idxf.to_broadcast(sh4), OP.is_equal)
            mval = small.tile(sh4, F32, tag="mval")
            nc.vector.tensor_tensor(
                mval, mask12,
                vals[:, :, 0:2].to_broadcast(sh4),
                OP.mult)
            wv = small.tile(sh4, F32, tag="wv")
            nc.vector.tensor_tensor(
                wv, mval,
                yg[:, :, E:3 * E].rearrange("p g (k e) -> p g k e", e=E),
                OP.mult)
            num = small.tile([P, GT, 1], F32, tag="num")
            nc.vector.tensor_reduce(num, wv, AX.XY, OP.add)
            # combined = num / (m1+m2)
            den = small.tile([P, GT, 1], F32, tag="den")
            nc.vector.tensor_tensor(den, vals[:, :, 0:1], vals[:, :, 1:2],
                                    OP.add)
            rec = small.tile([P, GT, 1], F32, tag="rec")
            nc.vector.reciprocal(rec, den)
            comb = small.tile([P, GT, 1], F32, tag="comb")
            nc.vector.tensor_tensor(comb, num, rec, OP.mult)

            # broadcast combined across O and store per tile: each tile's
            # 0.5 MB ships as soon as its broadcast finishes instead of
            # waiting for the whole block
            out_g = outs.tile([P, GT, O], F32)
            for t in range(GT):
                gt_i = ts + t
                if gt_i % 2 == 0:
                    nc.scalar.mul(out_g[:, t, :], ones, comb[:, t, :])
                else:
                    nc.vector.tensor_scalar_mul(out_g[:, t, :], ones,
                                                comb[:, t, :])
                o_eng = nc.scalar if gt_i % 2 == 0 else nc.sync
                o_eng.dma_start(out=ft_view[:, gt_i], in_=out_g[:, t, :])
            ts += GT

        nc.gpsimd.dma_start(
            out=idx[:].rearrange("(t p) k -> p t k", p=P), in_=idx_all)
    nc.finalize()
    return nc


def _prep_host_inputs(x, W_experts, b_experts, gate_W, gate_b, expert_biases):
    xf = np.asarray(x, dtype=np.float32).reshape(-1, D)
    Wc = np.concatenate([gate_W, W_experts[:, 0, :], W_experts[:, 1, :]],
                        axis=0).astype(np.float32)         # [24, D]
    # wct[p, c, m] = Wc[m, c*128 + p]
    wct = np.ascontiguousarray(
        Wc.T.reshape(KC, P, M).transpose(1, 0, 2)).astype(np.float32)
    bias24 = np.concatenate(
        [gate_b + expert_biases, b_experts[:, 0], b_experts[:, 1]]
    ).astype(np.float32)
    biast = np.ascontiguousarray(np.tile(bias24[None, :], (P, 1)))
    return xf, wct, biast


def make_in_maps(x, W_experts, b_experts, gate_W, gate_b, expert_biases):
    xf, wct, biast = _prep_host_inputs(
        x, W_experts, b_experts, gate_W, gate_b, expert_biases)
    maps = []
    for c in range(N_CORES):
        xT = xf[c * TOK:(c + 1) * TOK].T              # [D, TOK]
        x5 = xT.reshape(KC, P, NT, P).transpose(1, 2, 0, 3)
        maps.append({"xt": np.ascontiguousarray(x5),
                     "wct": wct, "biast": biast})
    return maps


def kernel(x, W_experts, b_experts, gate_W, gate_b, expert_biases):
    x = np.asarray(x, dtype=np.float32)
    W_experts = np.asarray(W_experts, dtype=np.float32)
    b_experts = np.asarray(b_experts, dtype=np.float32)
    gate_W = np.asarray(gate_W, dtype=np.float32)
    gate_b = np.asarray(gate_b, dtype=np.float32)
    expert_biases = np.asarray(expert_biases, dtype=np.float32)

    in_maps = make_in_maps(
        x, W_experts, b_experts, gate_W, gate_b, expert_biases)
    nc = build_bass()
    res = run_bass_kernel_spmd(nc, in_maps, list(range(N_CORES))).results
    final = np.concatenate([res[c]["final"] for c in range(N_CORES)], axis=0)
    idx = np.concatenate([res[c]["idx"] for c in range(N_CORES)], axis=0)
    return (final.reshape(B, S, O),
            idx.reshape(B, S, 2).astype(np.int32))
